# revision 4
# baseline (speedup 1.0000x reference)
"""GCN forward on 8 Trainium2 NeuronCores (Bass/Tile).

Strategy (dst-node sharding, graph/data parallel):
  - 8 cores each own 12500 destination nodes (padded to 12544 = 98 windows of 128).
  - Node order within a core is degree-sorted (perm) so per-window slot padding
    stays small; all per-window slot counts are shared across cores (SPMD: one
    program, per-core data).
  - Layer math is refactored so per-edge normalization disappears:
        out1 = relu(b1 + d * A_sum(d * (x@W1)));  d = deg^-1/2
        out2 = logsoftmax(b2 + d * (A_sum(d * h1) @ W2))
    where A_sum(u)[n] = sum_{e: dst=n} u[src_e] is a pure segment-sum, so both
    layers share ONE gather plan over 64-wide rows.
  - u (=d*x@W1) is computed from host-pretransposed x slices, AllGathered to a
    full table, then aggregated via bulk dma_gather calls (int16 indices,
    4 source ranges of 25088 rows each to fit the signed-int16 index space)
    and strided reduce_sum on the vector engine.
"""

import sys

sys.path.insert(0, "/opt/trn_rl_repo")

import numpy as np

N_NODES = 100000
N_EDGES = 1600000
N_FEAT = 512
HIDDEN = 64
N_CLASSES = 16
NC = 8

P = 128  # SBUF partitions
MAX_CALL_IDX = 8192  # dma_gather HW limit (single_packet=False)


def _default_cfg():
    return dict(
        n=N_NODES,
        feat=N_FEAT,
        hidden=HIDDEN,
        classes=N_CLASSES,
        nc=NC,
        npc=N_NODES // NC,  # real nodes per core
    )


def _derived(cfg):
    npc = cfg["npc"]
    nw = -(-npc // P)  # windows per core
    m = nw * P  # padded nodes per core
    r4 = 2 * m  # rows per gather range (2 cores' shards)
    assert r4 <= 32767, r4
    assert cfg["nc"] == 8
    return nw, m, r4


def build_plan(edge_index, deg_dtype=np.float32, cfg=None):
    """Host-side: permutations, dinv, shared window schedule, per-core plans."""
    cfg = cfg or _default_cfg()
    n, npc, nc = cfg["n"], cfg["npc"], cfg["nc"]
    nw, m, r4 = _derived(cfg)

    src = np.asarray(edge_index[0], dtype=np.int64)
    dst = np.asarray(edge_index[1], dtype=np.int64)
    loop = np.arange(n, dtype=np.int64)
    src_a = np.concatenate([src, loop])
    dst_a = np.concatenate([dst, loop])

    deg = np.bincount(dst_a, minlength=n).astype(deg_dtype)
    dinv = np.where(deg > 0, 1.0 / np.sqrt(deg), 0.0).astype(np.float32)

    # degree-sorted permutation per core; UPOS = row of each node in the
    # AllGathered table
    perms = []
    upos = np.empty(n, dtype=np.int64)
    for c in range(nc):
        d_c = deg[c * npc : (c + 1) * npc]
        pi = np.argsort(-d_c, kind="stable")
        perms.append(pi)
        upos[c * npc + pi] = c * m + np.arange(npc)

    # per-core edge lists grouped by (range k, dst position)
    core_edges = []
    for c in range(nc):
        sel = (dst_a >= c * npc) & (dst_a < (c + 1) * npc)
        s_c = src_a[sel]
        d_c = dst_a[sel] - c * npc
        pos = np.empty(npc, dtype=np.int64)
        pos[perms[c]] = np.arange(npc)
        pd = pos[d_c]
        r = upos[s_c]
        k = r // r4
        order = np.lexsort((r, pd, k))
        core_edges.append((k[order], pd[order], (r - k * r4)[order]))

    # shared per-(window, range) slot counts
    d1 = np.zeros((nw, 4), dtype=np.int64)
    counts = []
    for c in range(nc):
        k, pd, _ = core_edges[c]
        cnt = np.bincount(k * m + pd, minlength=4 * m).reshape(4, m)
        counts.append(cnt)
        for kk in range(4):
            d1[:, kk] = np.maximum(
                d1[:, kk], cnt[kk].reshape(nw, P).max(axis=1)
            )
    d1 = np.maximum(d1, 1)

    total_slots = int(d1.sum()) * P
    real_slots = sum(len(ce[0]) for ce in core_edges)
    pad_ratio = total_slots * nc / max(real_slots * 1.0, 1.0)

    # call schedule: greedy windows per range, <= MAX_CALL_IDX idxs per call
    # call = (k, w_start, w_end, col_off_list)
    calls = []
    for kk in range(4):
        wa = 0
        while wa < nw:
            tot = 0
            wb = wa
            while wb < nw and tot + P * d1[wb, kk] <= MAX_CALL_IDX:
                tot += P * int(d1[wb, kk])
                wb += 1
            assert wb > wa, (kk, wa, d1[wa, kk])
            calls.append((kk, wa, wb, tot))
            wa = wb

    # per-core gather index stream, wrapped-16 and replicated x8
    pad_local = npc  # first dummy row of the even core of each range pair
    assert npc < m, "pad rows require npc not divisible by 128"
    gplans = []
    for c in range(nc):
        k, pd, val = core_edges[c]
        cnt = counts[c]
        # slot index within (k, pos)
        gid = k * m + pd
        starts = np.zeros(4 * m + 1, dtype=np.int64)
        np.cumsum(np.bincount(gid, minlength=4 * m), out=starts[1:])
        slot = np.arange(len(gid)) - starts[gid]

        stream = np.full(total_slots, pad_local, dtype=np.int64)
        # offsets of each (k, w) block in the global stream
        blk_off = np.zeros((4, nw), dtype=np.int64)
        off = 0
        for kk, wa, wb, tot in calls:
            for w in range(wa, wb):
                blk_off[kk, w] = off
                off += P * int(d1[w, kk])
        assert off == total_slots
        w_of = pd // P
        p_of = pd % P
        pos_in_stream = blk_off[k, w_of] + slot * P + p_of
        stream[pos_in_stream] = val

        wrapped = np.zeros((P, total_slots // 16), dtype=np.int16)
        wrapped[:16, :] = stream.astype(np.int16).reshape(-1, 16).T
        wrapped[:] = np.tile(wrapped[:16, :], (8, 1))
        gplans.append(wrapped)

    # dinv tiles [P, nw] (partition-major per window) per core, dummies = 0
    dinv_tiles = []
    for c in range(nc):
        dv = np.zeros(m, dtype=np.float32)
        dv[: npc] = dinv[c * npc + perms[c]]
        dinv_tiles.append(np.ascontiguousarray(dv.reshape(nw, P).T))

    return dict(
        cfg=cfg,
        nw=nw,
        m=m,
        r4=r4,
        perms=perms,
        dinv=dinv,
        d1=d1,
        calls=calls,
        gplans=gplans,
        dinv_tiles=dinv_tiles,
        total_slots=total_slots,
        pad_ratio=pad_ratio,
    )


def build_program(plan):
    import concourse.bacc as bacc
    import concourse.mybir as mybir
    import concourse.tile as tile

    cfg = plan["cfg"]
    nw, m, r4 = plan["nw"], plan["m"], plan["r4"]
    feat, hid, ncls = cfg["feat"], cfg["hidden"], cfg["classes"]
    nc_cores = cfg["nc"]
    d1, calls, total_slots = plan["d1"], plan["calls"], plan["total_slots"]
    kf = feat // P  # feature chunks

    nc = bacc.Bacc(
        "TRN2",
        target_bir_lowering=False,
        debug=False,
        enable_asserts=True,
        num_devices=nc_cores,
    )
    f32 = mybir.dt.float32
    i16 = mybir.dt.int16

    xT = nc.dram_tensor("xT", [feat, m], f32, kind="ExternalInput")
    W1 = nc.dram_tensor("W1", [feat, hid], f32, kind="ExternalInput")
    W2 = nc.dram_tensor("W2", [hid, ncls], f32, kind="ExternalInput")
    b1b = nc.dram_tensor("b1b", [P, hid], f32, kind="ExternalInput")
    b2b = nc.dram_tensor("b2b", [P, ncls], f32, kind="ExternalInput")
    iden = nc.dram_tensor("iden", [P, P], f32, kind="ExternalInput")
    dinv_t = nc.dram_tensor("dinv_t", [P, nw], f32, kind="ExternalInput")
    gplan = nc.dram_tensor(
        "gplan", [P, total_slots // 16], i16, kind="ExternalInput"
    )
    out = nc.dram_tensor("out", [m, ncls], f32, kind="ExternalOutput")

    u_loc = nc.dram_tensor("u_loc", [m, hid], f32)
    w_loc = nc.dram_tensor("w_loc", [m, hid], f32)
    u_full = nc.dram_tensor("u_full", [nc_cores * m, hid], f32, addr_space="Shared")
    w_full = nc.dram_tensor("w_full", [nc_cores * m, hid], f32, addr_space="Shared")

    with tile.TileContext(nc) as tc:
        with (
            tc.tile_pool(name="const", bufs=1) as cpool,
            tc.tile_pool(name="xload", bufs=3) as xpool,
            tc.tile_pool(name="work", bufs=3) as wpool,
            tc.tile_pool(name="gath", bufs=3) as gpool,
            tc.tile_pool(name="idx", bufs=3) as ipool,
            tc.tile_pool(name="agg", bufs=1) as apool,
            tc.tile_pool(name="ps", bufs=3, space="PSUM") as pspool,
            tc.tile_pool(name="psv", bufs=2, space="PSUM") as pvpool,
            tc.tile_pool(name="pst", bufs=2, space="PSUM") as pstpool,
        ):
            w1c = cpool.tile([P, kf * hid], f32)
            for k in range(kf):
                nc.sync.dma_start(
                    out=w1c[:, k * hid : (k + 1) * hid],
                    in_=W1[k * P : (k + 1) * P, :],
                )
            w2t = cpool.tile([hid, ncls], f32)
            nc.sync.dma_start(out=w2t[:], in_=W2[:])
            b1t = cpool.tile([P, hid], f32)
            nc.sync.dma_start(out=b1t[:], in_=b1b[:])
            b2t = cpool.tile([P, ncls], f32)
            nc.sync.dma_start(out=b2t[:], in_=b2b[:])
            idt = cpool.tile([P, P], f32)
            nc.sync.dma_start(out=idt[:], in_=iden[:])
            dvt = cpool.tile([P, nw], f32)
            nc.sync.dma_start(out=dvt[:], in_=dinv_t[:])

            # ---------- phase A: u = d * (x @ W1), window by window ----------
            XB = 4  # windows per x-block
            for wb in range(0, nw, XB):
                we = min(wb + XB, nw)
                cols = (we - wb) * P
                xt = xpool.tile([P, kf * cols], f32, tag="xt")
                for k in range(kf):
                    nc.sync.dma_start(
                        out=xt[:, k * cols : k * cols + cols],
                        in_=xT[k * P : (k + 1) * P, wb * P : wb * P + cols],
                    )
                for w in range(wb, we):
                    ph = pspool.tile([P, hid], f32, tag="ph")
                    for k in range(kf):
                        nc.tensor.matmul(
                            ph[:],
                            lhsT=xt[:, k * cols + (w - wb) * P : k * cols + (w - wb + 1) * P],
                            rhs=w1c[:, k * hid : (k + 1) * hid],
                            start=(k == 0),
                            stop=(k == kf - 1),
                        )
                    ut = wpool.tile([P, hid], f32, tag="ut")
                    nc.vector.tensor_scalar_mul(ut[:], ph[:], dvt[:, w : w + 1])
                    nc.sync.dma_start(
                        out=u_loc[w * P : (w + 1) * P, :], in_=ut[:]
                    )

            cc_u = nc.gpsimd.collective_compute(
                "AllGather",
                mybir.AluOpType.bypass,
                replica_groups=[list(range(nc_cores))],
                ins=[u_loc[:]],
                outs=[u_full[:]],
            )

            # ---------- shared aggregation pass ----------
            def aggregation(table, cc_dep, tag):
                """agg_all[:, w*hid:(w+1)*hid] = sum over slots of table rows."""
                agg_all = apool.tile([P, nw * hid], f32, tag=f"agg{tag}")
                seen = set()
                off16 = 0
                for kk, wa, wb, tot in calls:
                    idx_t = ipool.tile([P, MAX_CALL_IDX // 16], i16, tag="idx")
                    nc.sync.dma_start(
                        out=idx_t[:, : tot // 16],
                        in_=gplan[:, off16 : off16 + tot // 16],
                    )
                    gt = gpool.tile([P, (MAX_CALL_IDX // P) * hid], f32, tag="gt")
                    g = nc.gpsimd.dma_gather(
                        out_ap=gt[:, : (tot // P) * hid].rearrange(
                            "p (c f) -> p c f", f=hid
                        ),
                        in_ap=table[kk * r4 : (kk + 1) * r4, :],
                        idxs_ap=idx_t[:, : tot // 16],
                        num_idxs=tot,
                        num_idxs_reg=tot,
                        elem_size=hid,
                        single_packet=False,
                    )
                    tile.add_dep_helper(g.ins, cc_dep.ins, reason="gather after AG")
                    col = 0
                    for w in range(wa, wb):
                        dw = int(d1[w, kk])
                        seg = gt[:, col * hid : (col + dw) * hid].rearrange(
                            "p (s f) -> p f s", f=hid
                        )
                        if w not in seen:
                            seen.add(w)
                            nc.vector.reduce_sum(
                                out=agg_all[:, w * hid : (w + 1) * hid],
                                in_=seg,
                                axis=mybir.AxisListType.X,
                            )
                        else:
                            tmp = wpool.tile([P, hid], f32, tag="rtmp")
                            nc.vector.reduce_sum(
                                out=tmp[:], in_=seg, axis=mybir.AxisListType.X
                            )
                            nc.vector.tensor_add(
                                out=agg_all[:, w * hid : (w + 1) * hid],
                                in0=agg_all[:, w * hid : (w + 1) * hid],
                                in1=tmp[:],
                            )
                        col += dw
                    off16 += tot // 16
                return agg_all

            agg1 = aggregation(u_full, cc_u, "1")

            # h1 = relu(d*agg1 + b1); w = d*h1 -> w_loc
            for w in range(nw):
                t1 = wpool.tile([P, hid], f32, tag="t1")
                nc.vector.tensor_scalar_mul(
                    t1[:], agg1[:, w * hid : (w + 1) * hid], dvt[:, w : w + 1]
                )
                nc.vector.tensor_add(out=t1[:], in0=t1[:], in1=b1t[:])
                h1 = wpool.tile([P, hid], f32, tag="h1")
                nc.scalar.activation(
                    h1[:], t1[:], mybir.ActivationFunctionType.Relu
                )
                wt = wpool.tile([P, hid], f32, tag="wt")
                nc.vector.tensor_scalar_mul(wt[:], h1[:], dvt[:, w : w + 1])
                nc.sync.dma_start(out=w_loc[w * P : (w + 1) * P, :], in_=wt[:])

            cc_w = nc.gpsimd.collective_compute(
                "AllGather",
                mybir.AluOpType.bypass,
                replica_groups=[list(range(nc_cores))],
                ins=[w_loc[:]],
                outs=[w_full[:]],
            )

            agg2 = aggregation(w_full, cc_w, "2")

            # out = logsoftmax(d*(agg2 @ W2) + b2)
            for w in range(nw):
                a2 = agg2[:, w * hid : (w + 1) * hid]
                pt = pstpool.tile([hid, P], f32, tag="pt")
                nc.tensor.transpose(out=pt[:], in_=a2, identity=idt[:])
                a2t = wpool.tile([hid, P], f32, tag="a2t")
                nc.vector.tensor_copy(out=a2t[:], in_=pt[:])
                pv = pvpool.tile([P, ncls], f32, tag="pv")
                nc.tensor.matmul(pv[:], lhsT=a2t[:], rhs=w2t[:], start=True, stop=True)
                z = wpool.tile([P, ncls], f32, tag="z")
                nc.vector.tensor_scalar_mul(z[:], pv[:], dvt[:, w : w + 1])
                nc.vector.tensor_add(out=z[:], in0=z[:], in1=b2t[:])
                mx = wpool.tile([P, 1], f32, tag="mx")
                nc.vector.reduce_max(out=mx[:], in_=z[:], axis=mybir.AxisListType.X)
                nc.vector.tensor_scalar_sub(z[:], z[:], mx[:])
                ez = wpool.tile([P, ncls], f32, tag="ez")
                nc.scalar.activation(ez[:], z[:], mybir.ActivationFunctionType.Exp)
                sm = wpool.tile([P, 1], f32, tag="sm")
                nc.vector.reduce_sum(out=sm[:], in_=ez[:], axis=mybir.AxisListType.X)
                lg = wpool.tile([P, 1], f32, tag="lg")
                nc.scalar.activation(lg[:], sm[:], mybir.ActivationFunctionType.Ln)
                nc.vector.tensor_scalar_sub(z[:], z[:], lg[:])
                nc.sync.dma_start(out=out[w * P : (w + 1) * P, :], in_=z[:])

    nc.compile()
    return nc


def run(inputs, cfg=None, plan=None, nc=None):
    from concourse import bass_utils

    cfg = cfg or _default_cfg()
    x = np.asarray(inputs["x"], dtype=np.float32)
    W1 = np.asarray(inputs["W1"], dtype=np.float32)
    b1 = np.asarray(inputs["b1"], dtype=np.float32)
    W2 = np.asarray(inputs["W2"], dtype=np.float32)
    b2 = np.asarray(inputs["b2"], dtype=np.float32)

    if plan is None:
        plan = build_plan(inputs["edge_index"], cfg=cfg)
    if nc is None:
        nc = build_program(plan)

    n, npc, nc_cores = cfg["n"], cfg["npc"], cfg["nc"]
    m = plan["m"]
    iden = np.eye(P, dtype=np.float32)
    b1b = np.tile(b1[None, :], (P, 1)).astype(np.float32)
    b2b = np.tile(b2[None, :], (P, 1)).astype(np.float32)

    in_maps = []
    for c in range(nc_cores):
        gperm = c * npc + plan["perms"][c]
        xT = np.zeros((cfg["feat"], m), dtype=np.float32)
        xT[:, :npc] = x[gperm].T
        in_maps.append(
            dict(
                xT=np.ascontiguousarray(xT),
                W1=W1,
                W2=W2,
                b1b=b1b,
                b2b=b2b,
                iden=iden,
                dinv_t=plan["dinv_tiles"][c],
                gplan=plan["gplans"][c],
            )
        )

    res = bass_utils.run_bass_kernel_spmd(
        nc, in_maps, core_ids=list(range(nc_cores))
    )

    outp = np.empty((n, cfg["classes"]), dtype=np.float32)
    for c in range(nc_cores):
        outp[c * npc + plan["perms"][c]] = res.results[c]["out"][:npc]
    return outp


def kernel(**inputs) -> np.ndarray:
    return run(inputs, cfg=_default_cfg())


# revision 8
# speedup vs baseline: 42.8243x; 42.8243x over previous
"""GCN forward on 8 Trainium2 NeuronCores (Bass/Tile).

Strategy (dst-node sharding, graph/data parallel):
  - 8 cores each own 12500 destination nodes (padded to 12544 = 98 windows of 128).
  - Node order within a core is degree-sorted (perm) so per-window slot padding
    stays small; all per-window slot counts are shared across cores (SPMD: one
    program, per-core data).
  - Layer math is refactored so per-edge normalization disappears:
        out1 = relu(b1 + d * A_sum(d * (x@W1)));  d = deg^-1/2
        out2 = logsoftmax(b2 + d * (A_sum(d * h1) @ W2))
    where A_sum(u)[n] = sum_{e: dst=n} u[src_e] is a pure segment-sum, so both
    layers share ONE gather plan over 64-wide rows.
  - u (=d*x@W1) is computed from host-pretransposed x slices, AllGathered to a
    full table, then aggregated via bulk dma_gather calls (int16 indices,
    4 source ranges of 25088 rows each to fit the signed-int16 index space)
    and strided reduce_sum on the vector engine.
"""

import sys

sys.path.insert(0, "/opt/trn_rl_repo")

import numpy as np

N_NODES = 100000
N_EDGES = 1600000
N_FEAT = 512
HIDDEN = 64
N_CLASSES = 16
NC = 8

P = 128  # SBUF partitions
MAX_CALL_IDX = 8192  # dma_gather HW limit (single_packet=False)


def _default_cfg():
    return dict(
        n=N_NODES,
        feat=N_FEAT,
        hidden=HIDDEN,
        classes=N_CLASSES,
        nc=NC,
        npc=N_NODES // NC,  # real nodes per core
    )


def _derived(cfg):
    npc = cfg["npc"]
    nw = -(-npc // P)  # windows per core
    m = nw * P  # padded nodes per core
    r4 = 2 * m  # rows per gather range (2 cores' shards)
    assert r4 <= 32767, r4
    assert cfg["nc"] == 8
    return nw, m, r4


def build_plan(edge_index, deg_dtype=np.float32, cfg=None):
    """Host-side: permutations, dinv, shared window schedule, per-core plans."""
    cfg = cfg or _default_cfg()
    n, npc, nc = cfg["n"], cfg["npc"], cfg["nc"]
    nw, m, r4 = _derived(cfg)

    src = np.asarray(edge_index[0], dtype=np.int64)
    dst = np.asarray(edge_index[1], dtype=np.int64)
    loop = np.arange(n, dtype=np.int64)
    src_a = np.concatenate([src, loop])
    dst_a = np.concatenate([dst, loop])

    deg = np.bincount(dst_a, minlength=n).astype(deg_dtype)
    dinv = np.where(deg > 0, 1.0 / np.sqrt(deg), 0.0).astype(np.float32)

    # degree-sorted permutation per core; UPOS = row of each node in the
    # AllGathered table
    perms = []
    upos = np.empty(n, dtype=np.int64)
    for c in range(nc):
        d_c = deg[c * npc : (c + 1) * npc]
        pi = np.argsort(-d_c, kind="stable")
        perms.append(pi)
        upos[c * npc + pi] = c * m + np.arange(npc)

    # per-core edge lists grouped by (range k, dst position)
    core_edges = []
    for c in range(nc):
        sel = (dst_a >= c * npc) & (dst_a < (c + 1) * npc)
        s_c = src_a[sel]
        d_c = dst_a[sel] - c * npc
        pos = np.empty(npc, dtype=np.int64)
        pos[perms[c]] = np.arange(npc)
        pd = pos[d_c]
        r = upos[s_c]
        k = r // r4
        order = np.lexsort((r, pd, k))
        core_edges.append((k[order], pd[order], (r - k * r4)[order]))

    # shared per-(window, range) slot counts
    d1 = np.zeros((nw, 4), dtype=np.int64)
    counts = []
    for c in range(nc):
        k, pd, _ = core_edges[c]
        cnt = np.bincount(k * m + pd, minlength=4 * m).reshape(4, m)
        counts.append(cnt)
        for kk in range(4):
            d1[:, kk] = np.maximum(
                d1[:, kk], cnt[kk].reshape(nw, P).max(axis=1)
            )
    d1 = np.maximum(d1, 1)

    total_slots = int(d1.sum()) * P
    real_slots = sum(len(ce[0]) for ce in core_edges)
    pad_ratio = total_slots * nc / max(real_slots * 1.0, 1.0)

    # call schedule: greedy windows per range, <= MAX_CALL_IDX idxs per call
    # call = (k, w_start, w_end, col_off_list)
    calls = []
    for kk in range(4):
        wa = 0
        while wa < nw:
            tot = 0
            wb = wa
            while wb < nw and tot + P * d1[wb, kk] <= MAX_CALL_IDX:
                tot += P * int(d1[wb, kk])
                wb += 1
            assert wb > wa, (kk, wa, d1[wa, kk])
            calls.append((kk, wa, wb, tot))
            wa = wb

    # per-core gather index stream, wrapped-16 and replicated x8
    pad_local = npc  # first dummy row of the even core of each range pair
    assert npc < m, "pad rows require npc not divisible by 128"
    gplans = []
    for c in range(nc):
        k, pd, val = core_edges[c]
        cnt = counts[c]
        # slot index within (k, pos)
        gid = k * m + pd
        starts = np.zeros(4 * m + 1, dtype=np.int64)
        np.cumsum(np.bincount(gid, minlength=4 * m), out=starts[1:])
        slot = np.arange(len(gid)) - starts[gid]

        stream = np.full(total_slots, pad_local, dtype=np.int64)
        # offsets of each (k, w) block in the global stream
        blk_off = np.zeros((4, nw), dtype=np.int64)
        off = 0
        for kk, wa, wb, tot in calls:
            for w in range(wa, wb):
                blk_off[kk, w] = off
                off += P * int(d1[w, kk])
        assert off == total_slots
        w_of = pd // P
        p_of = pd % P
        pos_in_stream = blk_off[k, w_of] + slot * P + p_of
        stream[pos_in_stream] = val

        wrapped = np.zeros((P, total_slots // 16), dtype=np.int16)
        wrapped[:16, :] = stream.astype(np.int16).reshape(-1, 16).T
        wrapped[:] = np.tile(wrapped[:16, :], (8, 1))
        gplans.append(wrapped)

    # dinv tiles [P, nw] (partition-major per window) per core, dummies = 0
    dinv_tiles = []
    for c in range(nc):
        dv = np.zeros(m, dtype=np.float32)
        dv[: npc] = dinv[c * npc + perms[c]]
        dinv_tiles.append(np.ascontiguousarray(dv.reshape(nw, P).T))

    return dict(
        cfg=cfg,
        nw=nw,
        m=m,
        r4=r4,
        perms=perms,
        dinv=dinv,
        d1=d1,
        calls=calls,
        gplans=gplans,
        dinv_tiles=dinv_tiles,
        total_slots=total_slots,
        pad_ratio=pad_ratio,
    )


def build_program(plan, no_cc=False):
    import concourse.bacc as bacc
    import concourse.mybir as mybir
    import concourse.tile as tile

    cfg = plan["cfg"]
    nw, m, r4 = plan["nw"], plan["m"], plan["r4"]
    feat, hid, ncls = cfg["feat"], cfg["hidden"], cfg["classes"]
    nc_cores = cfg["nc"]
    d1, calls, total_slots = plan["d1"], plan["calls"], plan["total_slots"]
    kf = feat // P  # feature chunks

    nc = bacc.Bacc(
        "TRN2",
        target_bir_lowering=False,
        debug=False,
        enable_asserts=True,
        num_devices=nc_cores,
    )
    f32 = mybir.dt.float32
    i16 = mybir.dt.int16

    xT = nc.dram_tensor("xT", [feat, m], f32, kind="ExternalInput")
    W1 = nc.dram_tensor("W1", [feat, hid], f32, kind="ExternalInput")
    W2 = nc.dram_tensor("W2", [hid, ncls], f32, kind="ExternalInput")
    b1b = nc.dram_tensor("b1b", [P, hid], f32, kind="ExternalInput")
    b2b = nc.dram_tensor("b2b", [P, ncls], f32, kind="ExternalInput")
    iden = nc.dram_tensor("iden", [P, P], f32, kind="ExternalInput")
    dinv_t = nc.dram_tensor("dinv_t", [P, nw], f32, kind="ExternalInput")
    gplan = nc.dram_tensor(
        "gplan", [P, total_slots // 16], i16, kind="ExternalInput"
    )
    out = nc.dram_tensor("out", [m, ncls], f32, kind="ExternalOutput")

    u_loc = nc.dram_tensor("u_loc", [m, hid], f32)
    w_loc = nc.dram_tensor("w_loc", [m, hid], f32)
    u_full = nc.dram_tensor("u_full", [nc_cores * m, hid], f32, addr_space="Shared")
    w_full = nc.dram_tensor("w_full", [nc_cores * m, hid], f32, addr_space="Shared")

    with tile.TileContext(nc) as tc:
        with (
            tc.tile_pool(name="const", bufs=1) as cpool,
            tc.tile_pool(name="xload", bufs=3) as xpool,
            tc.tile_pool(name="work", bufs=3) as wpool,
            tc.tile_pool(name="gath", bufs=3) as gpool,
            tc.tile_pool(name="idx", bufs=3) as ipool,
            tc.tile_pool(name="agg", bufs=1) as apool,
            tc.tile_pool(name="ps", bufs=3, space="PSUM") as pspool,
            tc.tile_pool(name="psv", bufs=2, space="PSUM") as pvpool,
            tc.tile_pool(name="pst", bufs=2, space="PSUM") as pstpool,
        ):
            w1c = cpool.tile([P, kf * hid], f32)
            for k in range(kf):
                nc.sync.dma_start(
                    out=w1c[:, k * hid : (k + 1) * hid],
                    in_=W1[k * P : (k + 1) * P, :],
                )
            w2t = cpool.tile([hid, ncls], f32)
            nc.sync.dma_start(out=w2t[:], in_=W2[:])
            b1t = cpool.tile([P, hid], f32)
            nc.sync.dma_start(out=b1t[:], in_=b1b[:])
            b2t = cpool.tile([P, ncls], f32)
            nc.sync.dma_start(out=b2t[:], in_=b2b[:])
            idt = cpool.tile([P, P], f32)
            nc.sync.dma_start(out=idt[:], in_=iden[:])
            dvt = cpool.tile([P, nw], f32)
            nc.sync.dma_start(out=dvt[:], in_=dinv_t[:])

            # ---------- phase A: u = d * (x @ W1), window by window ----------
            XB = 4  # windows per x-block
            for wb in range(0, nw, XB):
                we = min(wb + XB, nw)
                cols = (we - wb) * P
                xt = xpool.tile([P, kf * cols], f32, tag="xt")
                for k in range(kf):
                    nc.sync.dma_start(
                        out=xt[:, k * cols : k * cols + cols],
                        in_=xT[k * P : (k + 1) * P, wb * P : wb * P + cols],
                    )
                for w in range(wb, we):
                    ph = pspool.tile([P, hid], f32, tag="ph")
                    for k in range(kf):
                        nc.tensor.matmul(
                            ph[:],
                            lhsT=xt[:, k * cols + (w - wb) * P : k * cols + (w - wb + 1) * P],
                            rhs=w1c[:, k * hid : (k + 1) * hid],
                            start=(k == 0),
                            stop=(k == kf - 1),
                        )
                    ut = wpool.tile([P, hid], f32, tag="ut")
                    nc.vector.tensor_scalar_mul(ut[:], ph[:], dvt[:, w : w + 1])
                    nc.sync.dma_start(
                        out=u_loc[w * P : (w + 1) * P, :], in_=ut[:]
                    )

            if no_cc:
                cc_u = nc.sync.dma_start(out=u_full[: plan["m"], :], in_=u_loc[:])
            else:
                cc_u = nc.gpsimd.collective_compute(
                    "AllGather",
                    mybir.AluOpType.bypass,
                    replica_groups=[list(range(nc_cores))],
                    ins=[u_loc[:]],
                    outs=[u_full[:]],
                )

            # ---------- shared aggregation pass ----------
            def aggregation(table, cc_dep, tag):
                """agg_all[:, w*hid:(w+1)*hid] = sum over slots of table rows."""
                agg_all = apool.tile([P, nw * hid], f32, tag=f"agg{tag}")
                seen = set()
                off16 = 0
                for kk, wa, wb, tot in calls:
                    idx_t = ipool.tile([P, MAX_CALL_IDX // 16], i16, tag="idx")
                    nc.sync.dma_start(
                        out=idx_t[:, : tot // 16],
                        in_=gplan[:, off16 : off16 + tot // 16],
                    )
                    gt = gpool.tile([P, (MAX_CALL_IDX // P) * hid], f32, tag="gt")
                    g = nc.gpsimd.dma_gather(
                        out_ap=gt[:, : (tot // P) * hid].rearrange(
                            "p (c f) -> p c f", f=hid
                        ),
                        in_ap=table[kk * r4 : (kk + 1) * r4, :],
                        idxs_ap=idx_t[:, : tot // 16],
                        num_idxs=tot,
                        num_idxs_reg=tot,
                        elem_size=hid,
                        single_packet=False,
                    )
                    tile.add_dep_helper(g.ins, cc_dep.ins, reason="gather after AG")
                    col = 0
                    for w in range(wa, wb):
                        dw = int(d1[w, kk])
                        seg = gt[:, col * hid : (col + dw) * hid].rearrange(
                            "p (s f) -> p f s", f=hid
                        )
                        if w not in seen:
                            seen.add(w)
                            nc.vector.reduce_sum(
                                out=agg_all[:, w * hid : (w + 1) * hid],
                                in_=seg,
                                axis=mybir.AxisListType.X,
                            )
                        else:
                            tmp = wpool.tile([P, hid], f32, tag="rtmp")
                            nc.vector.reduce_sum(
                                out=tmp[:], in_=seg, axis=mybir.AxisListType.X
                            )
                            nc.vector.tensor_add(
                                out=agg_all[:, w * hid : (w + 1) * hid],
                                in0=agg_all[:, w * hid : (w + 1) * hid],
                                in1=tmp[:],
                            )
                        col += dw
                    off16 += tot // 16
                return agg_all

            agg1 = aggregation(u_full, cc_u, "1")

            # h1 = relu(d*agg1 + b1); w = d*h1 -> w_loc
            for w in range(nw):
                t1 = wpool.tile([P, hid], f32, tag="t1")
                nc.vector.tensor_scalar_mul(
                    t1[:], agg1[:, w * hid : (w + 1) * hid], dvt[:, w : w + 1]
                )
                nc.vector.tensor_add(out=t1[:], in0=t1[:], in1=b1t[:])
                h1 = wpool.tile([P, hid], f32, tag="h1")
                nc.scalar.activation(
                    h1[:], t1[:], mybir.ActivationFunctionType.Relu
                )
                wt = wpool.tile([P, hid], f32, tag="wt")
                nc.vector.tensor_scalar_mul(wt[:], h1[:], dvt[:, w : w + 1])
                nc.sync.dma_start(out=w_loc[w * P : (w + 1) * P, :], in_=wt[:])

            if no_cc:
                cc_w = nc.sync.dma_start(out=w_full[: plan["m"], :], in_=w_loc[:])
            else:
                cc_w = nc.gpsimd.collective_compute(
                    "AllGather",
                    mybir.AluOpType.bypass,
                    replica_groups=[list(range(nc_cores))],
                    ins=[w_loc[:]],
                    outs=[w_full[:]],
                )

            agg2 = aggregation(w_full, cc_w, "2")

            # out = logsoftmax(d*(agg2 @ W2) + b2)
            for w in range(nw):
                a2 = agg2[:, w * hid : (w + 1) * hid]
                pt = pstpool.tile([hid, P], f32, tag="pt")
                nc.tensor.transpose(out=pt[:], in_=a2, identity=idt[:])
                a2t = wpool.tile([hid, P], f32, tag="a2t")
                nc.vector.tensor_copy(out=a2t[:], in_=pt[:])
                pv = pvpool.tile([P, ncls], f32, tag="pv")
                nc.tensor.matmul(pv[:], lhsT=a2t[:], rhs=w2t[:], start=True, stop=True)
                z = wpool.tile([P, ncls], f32, tag="z")
                nc.vector.tensor_scalar_mul(z[:], pv[:], dvt[:, w : w + 1])
                nc.vector.tensor_add(out=z[:], in0=z[:], in1=b2t[:])
                mx = wpool.tile([P, 1], f32, tag="mx")
                nc.vector.reduce_max(out=mx[:], in_=z[:], axis=mybir.AxisListType.X)
                nc.vector.tensor_scalar_sub(z[:], z[:], mx[:])
                ez = wpool.tile([P, ncls], f32, tag="ez")
                nc.scalar.activation(ez[:], z[:], mybir.ActivationFunctionType.Exp)
                sm = wpool.tile([P, 1], f32, tag="sm")
                nc.vector.reduce_sum(out=sm[:], in_=ez[:], axis=mybir.AxisListType.X)
                lg = wpool.tile([P, 1], f32, tag="lg")
                nc.scalar.activation(lg[:], sm[:], mybir.ActivationFunctionType.Ln)
                nc.vector.tensor_scalar_sub(z[:], z[:], lg[:])
                nc.sync.dma_start(out=out[w * P : (w + 1) * P, :], in_=z[:])

    nc.compile()
    return nc


def make_in_maps(inputs, plan):
    cfg = plan["cfg"]
    x = np.asarray(inputs["x"], dtype=np.float32)
    W1 = np.asarray(inputs["W1"], dtype=np.float32)
    b1 = np.asarray(inputs["b1"], dtype=np.float32)
    W2 = np.asarray(inputs["W2"], dtype=np.float32)
    b2 = np.asarray(inputs["b2"], dtype=np.float32)
    npc, nc_cores = cfg["npc"], cfg["nc"]
    m = plan["m"]
    iden = np.eye(P, dtype=np.float32)
    b1b = np.tile(b1[None, :], (P, 1)).astype(np.float32)
    b2b = np.tile(b2[None, :], (P, 1)).astype(np.float32)

    in_maps = []
    for c in range(nc_cores):
        gperm = c * npc + plan["perms"][c]
        xT = np.zeros((cfg["feat"], m), dtype=np.float32)
        xT[:, :npc] = x[gperm].T
        in_maps.append(
            dict(
                xT=np.ascontiguousarray(xT),
                W1=W1,
                W2=W2,
                b1b=b1b,
                b2b=b2b,
                iden=iden,
                dinv_t=plan["dinv_tiles"][c],
                gplan=plan["gplans"][c],
            )
        )
    return in_maps


def run(inputs, cfg=None, plan=None, nc=None):
    from concourse import bass_utils

    cfg = cfg or _default_cfg()
    if plan is None:
        plan = build_plan(inputs["edge_index"], cfg=cfg)
    if nc is None:
        nc = build_program(plan)

    n, npc, nc_cores = cfg["n"], cfg["npc"], cfg["nc"]
    in_maps = make_in_maps(inputs, plan)

    res = bass_utils.run_bass_kernel_spmd(
        nc, in_maps, core_ids=list(range(nc_cores))
    )

    outp = np.empty((n, cfg["classes"]), dtype=np.float32)
    for c in range(nc_cores):
        outp[c * npc + plan["perms"][c]] = res.results[c]["out"][:npc]
    return outp


def kernel(**inputs) -> np.ndarray:
    return run(inputs, cfg=_default_cfg())


# revision 14
# speedup vs baseline: 43.2848x; 1.0108x over previous
"""GCN forward on 8 Trainium2 NeuronCores (Bass/Tile).

Strategy (dst-node sharding, graph/data parallel):
  - 8 cores each own 12500 destination nodes (padded to 12544 = 98 windows of
    128). Node order within a core is degree-sorted so per-window slot padding
    stays small; the per-window slot schedule is shared across cores (SPMD:
    one program, per-core data).
  - Layer math is refactored so per-edge normalization disappears:
        out1 = relu(b1 + d * A_sum(d * (x@W1)));  d = deg^-1/2
        out2 = logsoftmax(b2 + d * (A_sum(d * h1) @ W2))
    where A_sum(u)[n] = sum_{e: dst=n} u[src_e] is a pure segment-sum, so both
    layers share ONE gather plan over 64-wide rows.
  - u (=d*x@W1) is computed from host-pretransposed x slices and AllGathered
    into a full table. Aggregation gathers QUADS of rows (1KB elements,
    index = row//4, fits the signed-int16 index space of dma_gather) and
    reduces them on the vector engine with a per-slot one-hot mask that
    selects the right row of each quad.
"""

import sys

sys.path.insert(0, "/opt/trn_rl_repo")

import numpy as np

N_NODES = 100000
N_EDGES = 1600000
N_FEAT = 512
HIDDEN = 64
N_CLASSES = 16
NC = 8

P = 128  # SBUF partitions
MAX_CALL_IDX = 4096  # <= 8192 HW limit; 4096 keeps quad gather tiles at 32KB/partition


def _default_cfg():
    return dict(
        n=N_NODES,
        feat=N_FEAT,
        hidden=HIDDEN,
        classes=N_CLASSES,
        nc=NC,
        npc=N_NODES // NC,  # real nodes per core
    )


def _derived(cfg):
    npc = cfg["npc"]
    nw = -(-npc // P)  # windows per core
    m = nw * P  # padded nodes per core
    nq = cfg["nc"] * m // 4  # quad rows in the gathered table
    assert nq <= 32767, nq
    return nw, m, nq


def build_plan(edge_index, cfg=None):
    """Host-side: permutations, dinv, shared window schedule, per-core plans."""
    cfg = cfg or _default_cfg()
    n, npc, nc = cfg["n"], cfg["npc"], cfg["nc"]
    nw, m, nq = _derived(cfg)

    src = np.asarray(edge_index[0], dtype=np.int64)
    dst = np.asarray(edge_index[1], dtype=np.int64)
    loop = np.arange(n, dtype=np.int64)
    src_a = np.concatenate([src, loop])
    dst_a = np.concatenate([dst, loop])

    deg = np.bincount(dst_a, minlength=n)
    dinv = np.where(deg > 0, 1.0 / np.sqrt(deg), 0.0).astype(np.float32)

    # degree-sorted permutation per core; UPOS = row of each node in the
    # AllGathered table
    perms = []
    upos = np.empty(n, dtype=np.int64)
    for c in range(nc):
        d_c = deg[c * npc : (c + 1) * npc]
        pi = np.argsort(-d_c, kind="stable")
        perms.append(pi)
        upos[c * npc + pi] = c * m + np.arange(npc)

    # per-core edge lists grouped by dst position
    core_edges = []
    counts = []
    for c in range(nc):
        sel = (dst_a >= c * npc) & (dst_a < (c + 1) * npc)
        s_c = src_a[sel]
        d_c = dst_a[sel] - c * npc
        pos = np.empty(npc, dtype=np.int64)
        pos[perms[c]] = np.arange(npc)
        pd = pos[d_c]
        order = np.argsort(pd, kind="stable")
        core_edges.append((pd[order], upos[s_c[order]]))
        counts.append(np.bincount(pd, minlength=m))

    # shared per-window slot counts
    d1 = np.zeros(nw, dtype=np.int64)
    for c in range(nc):
        d1 = np.maximum(d1, counts[c].reshape(nw, P).max(axis=1))
    d1 = np.maximum(d1, 1)

    total_slots = int(d1.sum()) * P
    real_slots = sum(len(ce[0]) for ce in core_edges)
    pad_ratio = total_slots * nc / max(real_slots * 1.0, 1.0)

    # call schedule: column ranges of the concatenated slot stream; a call may
    # cover partial windows (reduce handles partials)
    total_cols = total_slots // P
    cc = MAX_CALL_IDX // P
    calls = [(ca, min(ca + cc, total_cols)) for ca in range(0, total_cols, cc)]

    blk_off = np.zeros(nw, dtype=np.int64)
    off = 0
    for w in range(nw):
        blk_off[w] = off
        off += P * int(d1[w])
    assert off == total_slots

    # per-core gather index stream (quad indices, wrapped-16, replicated x8)
    # and per-slot 4-wide one-hot masks
    gplans = []
    masks = []
    for c in range(nc):
        pd, r = core_edges[c]
        starts = np.zeros(m + 1, dtype=np.int64)
        np.cumsum(counts[c], out=starts[1:])
        slot = np.arange(len(pd)) - starts[pd]

        stream = np.zeros(total_slots, dtype=np.int64)  # pad: quad 0, mask 0
        mask = np.zeros((total_slots, 4), dtype=np.float32)
        pos_in_stream = blk_off[pd // P] + slot * P + (pd % P)
        stream[pos_in_stream] = r // 4
        mask[pos_in_stream, r % 4] = 1.0

        wrapped = np.zeros((P, total_slots // 16), dtype=np.int16)
        wrapped[:16, :] = stream.astype(np.int16).reshape(-1, 16).T
        wrapped[:] = np.tile(wrapped[:16, :], (8, 1))
        gplans.append(wrapped)

        # mask layout must match the gathered tile: slot i -> partition i%P,
        # column (i//P); per column, 4 mask values
        mask_t = np.ascontiguousarray(
            mask.reshape(total_slots // P, P, 4).transpose(1, 0, 2)
        ).reshape(P, (total_slots // P) * 4)
        masks.append(mask_t)

    # dinv tiles [P, nw] (partition-major per window) per core, dummies = 0
    dinv_tiles = []
    for c in range(nc):
        dv = np.zeros(m, dtype=np.float32)
        dv[:npc] = dinv[c * npc + perms[c]]
        dinv_tiles.append(np.ascontiguousarray(dv.reshape(nw, P).T))

    return dict(
        cfg=cfg,
        nw=nw,
        m=m,
        nq=nq,
        perms=perms,
        dinv=dinv,
        d1=d1,
        calls=calls,
        gplans=gplans,
        masks=masks,
        dinv_tiles=dinv_tiles,
        total_slots=total_slots,
        pad_ratio=pad_ratio,
    )


def build_program(plan, no_cc=False):
    import concourse.bacc as bacc
    import concourse.mybir as mybir
    import concourse.tile as tile

    cfg = plan["cfg"]
    nw, m, nq = plan["nw"], plan["m"], plan["nq"]
    feat, hid, ncls = cfg["feat"], cfg["hidden"], cfg["classes"]
    nc_cores = cfg["nc"]
    d1, calls, total_slots = plan["d1"], plan["calls"], plan["total_slots"]
    kf = feat // P  # feature chunks
    QE = 4 * hid  # elements per gathered quad

    nc = bacc.Bacc(
        "TRN2",
        target_bir_lowering=False,
        debug=False,
        enable_asserts=True,
        num_devices=nc_cores,
    )
    f32 = mybir.dt.float32
    i16 = mybir.dt.int16

    xT = nc.dram_tensor("xT", [feat, m], f32, kind="ExternalInput")
    W1 = nc.dram_tensor("W1", [feat, hid], f32, kind="ExternalInput")
    W2 = nc.dram_tensor("W2", [hid, ncls], f32, kind="ExternalInput")
    b1b = nc.dram_tensor("b1b", [P, hid], f32, kind="ExternalInput")
    b2b = nc.dram_tensor("b2b", [P, ncls], f32, kind="ExternalInput")
    iden = nc.dram_tensor("iden", [P, P], f32, kind="ExternalInput")
    dinv_t = nc.dram_tensor("dinv_t", [P, nw], f32, kind="ExternalInput")
    gplan = nc.dram_tensor(
        "gplan", [P, total_slots // 16], i16, kind="ExternalInput"
    )
    gmask = nc.dram_tensor(
        "gmask", [P, (total_slots // P) * 4], f32, kind="ExternalInput"
    )
    out = nc.dram_tensor("out", [m, ncls], f32, kind="ExternalOutput")

    u_loc = nc.dram_tensor("u_loc", [m, hid], f32)
    w_loc = nc.dram_tensor("w_loc", [m, hid], f32)
    u_full = nc.dram_tensor("u_full", [nc_cores * m, hid], f32, addr_space="Shared")
    w_full = nc.dram_tensor("w_full", [nc_cores * m, hid], f32, addr_space="Shared")

    with tile.TileContext(nc) as tc:
        with (
            tc.tile_pool(name="const", bufs=1) as cpool,
            tc.tile_pool(name="xload", bufs=3) as xpool,
            tc.tile_pool(name="work", bufs=3) as wpool,
            tc.tile_pool(name="gath", bufs=2) as gpool,
            tc.tile_pool(name="idx", bufs=2) as ipool,
            tc.tile_pool(name="msk", bufs=2) as mpool,
            tc.tile_pool(name="agg", bufs=1) as apool,
            tc.tile_pool(name="ps", bufs=3, space="PSUM") as pspool,
            tc.tile_pool(name="psv", bufs=2, space="PSUM") as pvpool,
            tc.tile_pool(name="pst", bufs=2, space="PSUM") as pstpool,
        ):
            w1c = cpool.tile([P, kf * hid], f32)
            for k in range(kf):
                nc.sync.dma_start(
                    out=w1c[:, k * hid : (k + 1) * hid],
                    in_=W1[k * P : (k + 1) * P, :],
                )
            w2t = cpool.tile([hid, ncls], f32)
            nc.sync.dma_start(out=w2t[:], in_=W2[:])
            b1t = cpool.tile([P, hid], f32)
            nc.sync.dma_start(out=b1t[:], in_=b1b[:])
            b2t = cpool.tile([P, ncls], f32)
            nc.sync.dma_start(out=b2t[:], in_=b2b[:])
            idt = cpool.tile([P, P], f32)
            nc.sync.dma_start(out=idt[:], in_=iden[:])
            dvt = cpool.tile([P, nw], f32)
            nc.sync.dma_start(out=dvt[:], in_=dinv_t[:])

            # ---------- phase A: u = d * (x @ W1), window by window ----------
            XB = 4  # windows per x-block
            for wb in range(0, nw, XB):
                we = min(wb + XB, nw)
                cols = (we - wb) * P
                xt = xpool.tile([P, kf * cols], f32, tag="xt")
                for k in range(kf):
                    nc.sync.dma_start(
                        out=xt[:, k * cols : k * cols + cols],
                        in_=xT[k * P : (k + 1) * P, wb * P : wb * P + cols],
                    )
                for w in range(wb, we):
                    ph = pspool.tile([P, hid], f32, tag="ph")
                    for k in range(kf):
                        nc.tensor.matmul(
                            ph[:],
                            lhsT=xt[
                                :,
                                k * cols
                                + (w - wb) * P : k * cols
                                + (w - wb + 1) * P,
                            ],
                            rhs=w1c[:, k * hid : (k + 1) * hid],
                            start=(k == 0),
                            stop=(k == kf - 1),
                        )
                    ut = wpool.tile([P, hid], f32, tag="ut")
                    nc.vector.tensor_scalar_mul(ut[:], ph[:], dvt[:, w : w + 1])
                    nc.sync.dma_start(
                        out=u_loc[w * P : (w + 1) * P, :], in_=ut[:]
                    )

            def allgather(loc, full):
                if no_cc:
                    for rr in range(nc_cores):
                        cc = nc.sync.dma_start(
                            out=full[rr * m : (rr + 1) * m, :], in_=loc[:]
                        )
                    return cc
                return nc.gpsimd.collective_compute(
                    "AllGather",
                    mybir.AluOpType.bypass,
                    replica_groups=[list(range(nc_cores))],
                    ins=[loc[:]],
                    outs=[full[:]],
                )

            cc_u = allgather(u_loc, u_full)

            # ---------- shared aggregation pass ----------
            blk_col = np.cumsum(np.concatenate([[0], np.asarray(d1)]))

            def aggregation(table, cc_dep, tag):
                """agg_all[:, w*hid:(w+1)*hid] = masked sum of gathered quads."""
                agg_all = apool.tile([P, nw * hid], f32, tag=f"agg{tag}")
                tquad = table[:].rearrange("(q x) f -> q (x f)", x=4)
                seen = set()
                for ca, cb in calls:
                    cols = cb - ca
                    tot = cols * P
                    idx_t = ipool.tile([P, MAX_CALL_IDX // 16], i16, tag="idx")
                    nc.sync.dma_start(
                        out=idx_t[:, : tot // 16],
                        in_=gplan[:, ca * P // 16 : cb * P // 16],
                    )
                    mk_t = mpool.tile([P, (MAX_CALL_IDX // P) * 4], f32, tag="mk")
                    nc.sync.dma_start(
                        out=mk_t[:, : cols * 4],
                        in_=gmask[:, ca * 4 : cb * 4],
                    )
                    gt = gpool.tile([P, (MAX_CALL_IDX // P) * QE], f32, tag="gt")
                    g = nc.gpsimd.dma_gather(
                        out_ap=gt[:, : cols * QE].rearrange(
                            "p (c f) -> p c f", f=QE
                        ),
                        in_ap=tquad,
                        idxs_ap=idx_t[:, : tot // 16],
                        num_idxs=tot,
                        num_idxs_reg=tot,
                        elem_size=QE,
                        single_packet=False,
                    )
                    tile.add_dep_helper(g.ins, cc_dep.ins, reason="gather after AG")
                    # mask multiply in place: gt[p, s, f] *= mk[p, s] for the
                    # 4*cols sub-slots of hid elements each
                    nc.vector.tensor_tensor(
                        out=gt[:, : cols * QE].rearrange(
                            "p (s f) -> p s f", f=hid
                        ),
                        in0=gt[:, : cols * QE].rearrange(
                            "p (s f) -> p s f", f=hid
                        ),
                        in1=mk_t[:, : cols * 4]
                        .rearrange("p (s o) -> p s o", o=1)
                        .to_broadcast([P, cols * 4, hid]),
                        op=mybir.AluOpType.mult,
                    )
                    # reduce every window (sub)range inside [ca, cb)
                    for w in range(nw):
                        w0, w1 = int(blk_col[w]), int(blk_col[w + 1])
                        s0, s1 = max(w0, ca), min(w1, cb)
                        if s0 >= s1:
                            continue
                        seg = gt[
                            :, (s0 - ca) * QE : (s1 - ca) * QE
                        ].rearrange("p (s f) -> p f s", f=hid)
                        if w not in seen:
                            seen.add(w)
                            nc.vector.reduce_sum(
                                out=agg_all[:, w * hid : (w + 1) * hid],
                                in_=seg,
                                axis=mybir.AxisListType.X,
                            )
                        else:
                            tmp = wpool.tile([P, hid], f32, tag="rtmp")
                            nc.vector.reduce_sum(
                                out=tmp[:], in_=seg, axis=mybir.AxisListType.X
                            )
                            nc.vector.tensor_add(
                                out=agg_all[:, w * hid : (w + 1) * hid],
                                in0=agg_all[:, w * hid : (w + 1) * hid],
                                in1=tmp[:],
                            )
                return agg_all

            agg1 = aggregation(u_full, cc_u, "1")

            # h1 = relu(d*agg1 + b1); w = d*h1 -> w_loc
            for w in range(nw):
                t1 = wpool.tile([P, hid], f32, tag="t1")
                nc.vector.tensor_scalar_mul(
                    t1[:], agg1[:, w * hid : (w + 1) * hid], dvt[:, w : w + 1]
                )
                nc.vector.tensor_add(out=t1[:], in0=t1[:], in1=b1t[:])
                h1 = wpool.tile([P, hid], f32, tag="h1")
                nc.scalar.activation(
                    h1[:], t1[:], mybir.ActivationFunctionType.Relu
                )
                wt = wpool.tile([P, hid], f32, tag="wt")
                nc.vector.tensor_scalar_mul(wt[:], h1[:], dvt[:, w : w + 1])
                nc.sync.dma_start(out=w_loc[w * P : (w + 1) * P, :], in_=wt[:])

            cc_w = allgather(w_loc, w_full)

            agg2 = aggregation(w_full, cc_w, "2")

            # out = logsoftmax(d*(agg2 @ W2) + b2)
            for w in range(nw):
                a2 = agg2[:, w * hid : (w + 1) * hid]
                pt = pstpool.tile([hid, P], f32, tag="pt")
                nc.tensor.transpose(out=pt[:], in_=a2, identity=idt[:])
                a2t = wpool.tile([hid, P], f32, tag="a2t")
                nc.vector.tensor_copy(out=a2t[:], in_=pt[:])
                pv = pvpool.tile([P, ncls], f32, tag="pv")
                nc.tensor.matmul(
                    pv[:], lhsT=a2t[:], rhs=w2t[:], start=True, stop=True
                )
                z = wpool.tile([P, ncls], f32, tag="z")
                nc.vector.tensor_scalar_mul(z[:], pv[:], dvt[:, w : w + 1])
                nc.vector.tensor_add(out=z[:], in0=z[:], in1=b2t[:])
                mx = wpool.tile([P, 1], f32, tag="mx")
                nc.vector.reduce_max(
                    out=mx[:], in_=z[:], axis=mybir.AxisListType.X
                )
                nc.vector.tensor_scalar_sub(z[:], z[:], mx[:])
                ez = wpool.tile([P, ncls], f32, tag="ez")
                nc.scalar.activation(
                    ez[:], z[:], mybir.ActivationFunctionType.Exp
                )
                sm = wpool.tile([P, 1], f32, tag="sm")
                nc.vector.reduce_sum(
                    out=sm[:], in_=ez[:], axis=mybir.AxisListType.X
                )
                lg = wpool.tile([P, 1], f32, tag="lg")
                nc.scalar.activation(
                    lg[:], sm[:], mybir.ActivationFunctionType.Ln
                )
                nc.vector.tensor_scalar_sub(z[:], z[:], lg[:])
                nc.sync.dma_start(out=out[w * P : (w + 1) * P, :], in_=z[:])

    nc.compile()
    return nc


def make_in_maps(inputs, plan):
    cfg = plan["cfg"]
    x = np.asarray(inputs["x"], dtype=np.float32)
    W1 = np.asarray(inputs["W1"], dtype=np.float32)
    b1 = np.asarray(inputs["b1"], dtype=np.float32)
    W2 = np.asarray(inputs["W2"], dtype=np.float32)
    b2 = np.asarray(inputs["b2"], dtype=np.float32)
    npc, nc_cores = cfg["npc"], cfg["nc"]
    m = plan["m"]
    iden = np.eye(P, dtype=np.float32)
    b1b = np.tile(b1[None, :], (P, 1)).astype(np.float32)
    b2b = np.tile(b2[None, :], (P, 1)).astype(np.float32)

    in_maps = []
    for c in range(nc_cores):
        gperm = c * npc + plan["perms"][c]
        xT = np.zeros((cfg["feat"], m), dtype=np.float32)
        xT[:, :npc] = x[gperm].T
        in_maps.append(
            dict(
                xT=np.ascontiguousarray(xT),
                W1=W1,
                W2=W2,
                b1b=b1b,
                b2b=b2b,
                iden=iden,
                dinv_t=plan["dinv_tiles"][c],
                gplan=plan["gplans"][c],
                gmask=plan["masks"][c],
            )
        )
    return in_maps


def run(inputs, cfg=None, plan=None, nc=None):
    from concourse import bass_utils

    cfg = cfg or _default_cfg()
    if plan is None:
        plan = build_plan(inputs["edge_index"], cfg=cfg)
    if nc is None:
        nc = build_program(plan)

    n, npc, nc_cores = cfg["n"], cfg["npc"], cfg["nc"]
    in_maps = make_in_maps(inputs, plan)

    res = bass_utils.run_bass_kernel_spmd(
        nc, in_maps, core_ids=list(range(nc_cores))
    )

    outp = np.empty((n, cfg["classes"]), dtype=np.float32)
    for c in range(nc_cores):
        outp[c * npc + plan["perms"][c]] = res.results[c]["out"][:npc]
    return outp


def kernel(**inputs) -> np.ndarray:
    return run(inputs, cfg=_default_cfg())
